# revision 1
# baseline (speedup 1.0000x reference)
"""GQA attention kernel for Trainium2, 8 NeuronCores.

Sharding: core c -> (batch = c // 4, head-group g = c % 4).
Each core handles one batch and 8 contiguous Q heads (= 2 KV heads),
computes its slice of Q/K/V projections, RoPE, causal attention, and a
partial output projection (rows g*512:(g+1)*512 of Wo). Host sums the 4
partials per batch.

Layout trick: everything is computed transposed. Host passes x^T per
batch so projections run as W^T-stationary matmuls producing Q^T/K^T/V^T
([feature, token]) directly, which is exactly the layout the scores
matmul needs (contraction over head_dim on partitions). Scores are
computed transposed (S^T[j,i], keys on partitions) so the context
matmul can consume exp(S^T) directly with V as the stationary operand.
A ones-column appended to V yields the softmax denominators for free in
the same PSUM accumulation.
"""

import sys
import math

for _p in ("/opt/trn_rl_repo",):
    if _p not in sys.path:
        sys.path.append(_p)

import numpy as np
import ml_dtypes

import concourse.bass as bass
from concourse import bacc
import concourse.mybir as mybir
import concourse.tile as tile
from concourse.bass_utils import run_bass_kernel_spmd
from concourse.masks import make_identity

BF16 = mybir.dt.bfloat16
F32 = mybir.dt.float32

B, S, D = 2, 2048, 2048
NH, NKV, HD = 32, 8, 64
GROUP = NH // NKV          # 4 q heads per kv head
NCORES = 8
CPB = NCORES // B          # 4 cores per batch
HPC = NH // CPB            # 8 q heads per core
KVPC = NKV // CPB          # 2 kv heads per core
QW = HPC * HD              # 512 projected q cols per core
KW = KVPC * HD             # 128 projected kv cols per core

NT = S // 128              # 16 seq tiles of 128
NBL = S // 512             # 4 seq blocks of 512
KT = D // 128              # 16 contraction tiles
QF = QW // 128             # 4 row-tiles of Q^T

_nc_cache = None


def _build():
    nc = bacc.Bacc()
    xT = nc.dram_tensor("xT", [D, S], BF16, kind="ExternalInput")
    wq = nc.dram_tensor("wq", [D, QW], BF16, kind="ExternalInput")
    wk = nc.dram_tensor("wk", [D, KW], BF16, kind="ExternalInput")
    wv = nc.dram_tensor("wv", [D, KW], BF16, kind="ExternalInput")
    wo = nc.dram_tensor("wo", [QW, D], BF16, kind="ExternalInput")
    cos2 = nc.dram_tensor("cos2", [128, S], F32, kind="ExternalInput")
    sinm = nc.dram_tensor("sinm", [128, S], F32, kind="ExternalInput")
    msk = nc.dram_tensor("msk", [GROUP, 128, 512], BF16, kind="ExternalInput")
    out = nc.dram_tensor("out", [S, D], F32, kind="ExternalOutput")

    Exp = mybir.ActivationFunctionType.Exp

    with tile.TileContext(nc) as tc:
        with (
            tc.tile_pool(name="persist", bufs=1) as pp,
            tc.tile_pool(name="psum", bufs=8, space="PSUM") as psp,
        ):
            # ---- persistent tiles ----
            Qb = [pp.tile([128, S], BF16, name=f"qb{f}", tag=f"qb{f}") for f in range(QF)]
            Kb = pp.tile([128, S], BF16, name="kb", tag="kb")
            Vaug = [pp.tile([128, NT, 65], BF16, name=f"vaug{k}", tag=f"vaug{k}") for k in range(KVPC)]
            ctxT = [pp.tile([128, S], BF16, name=f"ctxt{f}", tag=f"ctxt{f}") for f in range(QF)]
            mskt = [pp.tile([128, 512], BF16, name=f"msk{k}", tag=f"msk{k}") for k in range(GROUP)]
            ident = pp.tile([128, 128], BF16, name="ident", tag="ident")

            make_identity(nc, ident[:])
            for k in range(GROUP):
                nc.sync.dma_start(mskt[k][:], msk[k, :, :])
            for k in range(KVPC):
                nc.gpsimd.memset(Vaug[k][:, :, 64:65], 1.0)

            # ==== phase 1+2: load x/weights, projections + rope ====
            with (
                tc.tile_pool(name="proj", bufs=1) as jp,
                tc.tile_pool(name="rope", bufs=3) as rp,
            ):
                xt = [jp.tile([128, S], BF16, name=f"xt{k}", tag=f"xt{k}") for k in range(KT)]
                wqt = [jp.tile([128, QW], BF16, name=f"wqt{k}", tag=f"wqt{k}") for k in range(KT)]
                wkt = [jp.tile([128, KW], BF16, name=f"wkt{k}", tag=f"wkt{k}") for k in range(KT)]
                wvt = [jp.tile([128, KW], BF16, name=f"wvt{k}", tag=f"wvt{k}") for k in range(KT)]
                cos2t = jp.tile([128, S], F32, name="cos2t", tag="cos2t")
                sinmt = jp.tile([128, S], F32, name="sinmt", tag="sinmt")
                VtT = jp.tile([128, S], BF16, name="vtt", tag="vtt")

                for k in range(KT):
                    nc.sync.dma_start(xt[k][:], xT[k * 128:(k + 1) * 128, :])
                    nc.sync.dma_start(wqt[k][:], wq[k * 128:(k + 1) * 128, :])
                    nc.sync.dma_start(wkt[k][:], wk[k * 128:(k + 1) * 128, :])
                    nc.sync.dma_start(wvt[k][:], wv[k * 128:(k + 1) * 128, :])
                nc.sync.dma_start(cos2t[:], cos2[:, :])
                nc.sync.dma_start(sinmt[:], sinm[:, :])

                def rope_store(ps, dst, tcol):
                    # ps: psum [128, 512] f32 holding raw Q^T/K^T rows.
                    # dst[:, tcol:tcol+512] <- rope(ps) in bf16.
                    qf = rp.tile([128, 512], F32, name="ropecp", tag="ropecp")
                    nc.scalar.copy(qf[:], ps[:])
                    rot = rp.tile([128, 512], F32, name="roperot", tag="roperot")
                    for base in (0, 64):
                        nc.sync.dma_start(rot[base:base + 32, :],
                                          qf[base + 32:base + 64, :])
                        nc.sync.dma_start(rot[base + 32:base + 64, :],
                                          qf[base:base + 32, :])
                    a = rp.tile([128, 512], F32, name="ropea", tag="ropea")
                    b = rp.tile([128, 512], F32, name="ropeb", tag="ropeb")
                    nc.vector.tensor_mul(a[:], qf[:], cos2t[:, tcol:tcol + 512])
                    nc.vector.tensor_mul(b[:], rot[:], sinmt[:, tcol:tcol + 512])
                    nc.vector.tensor_add(dst[:, tcol:tcol + 512], a[:], b[:])

                # Q^T: rows f*128.. of [QW, S]
                for f in range(QF):
                    for t in range(NBL):
                        ps = psp.tile([128, 512], F32, name="bank", tag="bank")
                        for k in range(KT):
                            nc.tensor.matmul(
                                ps[:], wqt[k][:, f * 128:(f + 1) * 128],
                                xt[k][:, t * 512:(t + 1) * 512],
                                start=(k == 0), stop=(k == KT - 1))
                        rope_store(ps, Qb[f], t * 512)

                # K^T: [128, S]
                for t in range(NBL):
                    ps = psp.tile([128, 512], F32, name="bank", tag="bank")
                    for k in range(KT):
                        nc.tensor.matmul(
                            ps[:], wkt[k][:], xt[k][:, t * 512:(t + 1) * 512],
                            start=(k == 0), stop=(k == KT - 1))
                    rope_store(ps, Kb, t * 512)

                # V^T: [128, S] (no rope)
                for t in range(NBL):
                    ps = psp.tile([128, 512], F32, name="bank", tag="bank")
                    for k in range(KT):
                        nc.tensor.matmul(
                            ps[:], wvt[k][:], xt[k][:, t * 512:(t + 1) * 512],
                            start=(k == 0), stop=(k == KT - 1))
                    nc.scalar.copy(VtT[:, t * 512:(t + 1) * 512], ps[:])

                # V_aug[kv][:, j, 0:64] = V^T[kv rows, j block].T ; col 64 = 1.0
                for kv in range(KVPC):
                    for j in range(NT):
                        tp = psp.tile([128, 512], BF16, name="bank", tag="bank")
                        kb = kv * 64
                        nc.tensor.transpose(
                            tp[0:128, 0:64],
                            VtT[kb:kb + 64, j * 128:(j + 1) * 128],
                            ident[kb:kb + 64, kb:kb + 64])
                        nc.scalar.copy(Vaug[kv][:, j, 0:64], tp[0:128, 0:64])

            # ==== phase 3: attention per local q head ====
            with (
                tc.tile_pool(name="attn", bufs=6) as ap,
                tc.tile_pool(name="smal", bufs=3) as sp,
                tc.tile_pool(name="wout", bufs=1) as wp,
                tc.tile_pool(name="ostg", bufs=2) as op,
            ):
                wot = [wp.tile([128, D], BF16, name=f"wot{c}", tag=f"wot{c}") for c in range(QF)]
                for c in range(QF):
                    nc.sync.dma_start(wot[c][:], wo[c * 128:(c + 1) * 128, :])

                # Q tiles are head-permuted (host): tile f holds local
                # heads f (kv0, rows 0:64) and f+4 (kv1, rows 64:128), so
                # the K lhsT base partition always matches the Q rhs base.
                def norm_ctx(cp_ib, fq, qr, ib):
                    # normalize as soon as this i-block's accumulation is
                    # done: frees the PSUM bank early and overlaps the
                    # slow 1-lane reciprocal with remaining PE work.
                    rc = sp.tile([1, 512], F32, name="rc", tag="rc")
                    nc.vector.reciprocal(rc[0:1, :], cp_ib[64:65, :])
                    bc = sp.tile([64, 512], F32, name="bc", tag="bc")
                    nc.gpsimd.partition_broadcast(bc[0:64, :], rc[0:1, :])
                    nc.vector.tensor_mul(
                        ctxT[fq][qr:qr + 64, ib * 512:(ib + 1) * 512],
                        cp_ib[0:64, :], bc[0:64, :])

                for hl in range(HPC):
                    kv = hl // GROUP
                    fq = hl % GROUP
                    qr = kv * 64
                    cp = [psp.tile([128, 512], F32, name="bank", tag="bank")
                          for _ in range(NBL)]
                    for j in range(NT):
                        for ib in range(j // 4, NBL):
                            # columns i < 128*koff of a diagonal tile are
                            # fully masked (zero contribution): restrict
                            # the stream to the valid column range.
                            koff = j - 4 * ib
                            c0 = 128 * koff if koff > 0 else 0
                            st = psp.tile([128, 512], F32, name="bank", tag="bank")
                            nc.tensor.matmul(
                                st[:, c0:512],
                                Kb[kv * 64:(kv + 1) * 64,
                                   j * 128:(j + 1) * 128],
                                Qb[fq][qr:qr + 64,
                                       ib * 512 + c0:(ib + 1) * 512],
                                start=True, stop=True)
                            pt = ap.tile([128, 512], BF16, name="pt", tag="pt")
                            nc.scalar.activation(pt[:, c0:512], st[:, c0:512],
                                                 Exp, scale=0.125)
                            if koff >= 0:
                                nc.vector.tensor_mul(pt[:, c0:512],
                                                     pt[:, c0:512],
                                                     mskt[koff][:, c0:512])
                            nc.tensor.matmul(
                                cp[ib][0:65, c0:512], Vaug[kv][:, j, :],
                                pt[:, c0:512],
                                start=(j == 0), stop=(j == 4 * ib + 3),
                                skip_group_check=True)
                        if j % 4 == 3:
                            ibd = j // 4
                            norm_ctx(cp[ibd], fq, qr, ibd)

                # ==== phase 4: partial out = ctx @ Wo_slice ====
                for t in range(NT):
                    ops = [psp.tile([128, 512], F32, name="bank", tag="bank")
                           for _ in range(NBL)]
                    for c in range(QF):
                        for o in range(NBL):
                            nc.tensor.matmul(
                                ops[o][:],
                                ctxT[c][:, t * 128:(t + 1) * 128],
                                wot[c][:, o * 512:(o + 1) * 512],
                                start=(c == 0), stop=(c == QF - 1))
                    ob = op.tile([128, D], F32, name="ob", tag="ob")
                    for o in range(NBL):
                        nc.vector.tensor_copy(ob[:, o * 512:(o + 1) * 512],
                                              ops[o][:])
                    nc.sync.dma_start(out[t * 128:(t + 1) * 128, :], ob[:])

    nc.finalize()
    return nc


def _get_nc():
    global _nc_cache
    if _nc_cache is None:
        _nc_cache = _build()
    return _nc_cache


def _prep_inputs(x, cos, sin, Wq, Wk, Wv, Wo):
    bf = ml_dtypes.bfloat16
    cosT = np.ascontiguousarray(cos.T.astype(np.float32))          # [64, S]
    sinT = sin.T.astype(np.float32)
    sinm64 = np.concatenate([-sinT[:32], sinT[32:]], axis=0)       # [64, S]
    cos2 = np.ascontiguousarray(np.concatenate([cosT, cosT], 0))   # [128, S]
    sinm = np.ascontiguousarray(np.concatenate([sinm64, sinm64], 0))
    msk = np.stack([
        (np.arange(128)[:, None] <= (np.arange(512)[None, :] - 128 * k))
        for k in range(GROUP)
    ]).astype(bf)                                                  # [4,128,512]

    # head permutation: Q^T tile f holds local heads (f, f+4) so that the
    # kv0/kv1 row base of K matches the q row base (PE base-partition rule)
    perm = [0, 4, 1, 5, 2, 6, 3, 7]
    colperm = np.concatenate(
        [np.arange(HD) + p * HD for p in perm])          # [QW]
    in_maps = []
    for c in range(NCORES):
        b, g = c // CPB, c % CPB
        xTb = np.ascontiguousarray(x[b].T.astype(bf))
        wq_g = Wq[:, g * QW:(g + 1) * QW][:, colperm]
        wo_g = Wo[g * QW:(g + 1) * QW, :][colperm, :]
        in_maps.append({
            "xT": xTb,
            "wq": np.ascontiguousarray(wq_g.astype(bf)),
            "wk": np.ascontiguousarray(Wk[:, g * KW:(g + 1) * KW].astype(bf)),
            "wv": np.ascontiguousarray(Wv[:, g * KW:(g + 1) * KW].astype(bf)),
            "wo": np.ascontiguousarray(wo_g.astype(bf)),
            "cos2": cos2,
            "sinm": sinm,
            "msk": msk,
        })
    return in_maps


def kernel(x, mask, cos, sin, Wq, Wk, Wv, Wo, _trace=False, **kw):
    x = np.asarray(x, dtype=np.float32)
    in_maps = _prep_inputs(x, np.asarray(cos), np.asarray(sin),
                           np.asarray(Wq), np.asarray(Wk),
                           np.asarray(Wv), np.asarray(Wo))
    nc = _get_nc()
    res = run_bass_kernel_spmd(nc, in_maps, core_ids=list(range(NCORES)),
                               trace=_trace, **kw)
    parts = [np.asarray(r["out"], dtype=np.float32) for r in res.results]
    full = np.stack([
        sum(parts[b * CPB + g] for g in range(CPB)) for b in range(B)
    ]).astype(np.float32)
    if _trace:
        kernel.last_result = res
    return full



# revision 12
# speedup vs baseline: 1.1667x; 1.1667x over previous
"""GQA attention kernel for Trainium2, 8 NeuronCores.

Sharding: core c -> (batch = c // 4, head-group g = c % 4).
Each core handles one batch and 8 contiguous Q heads (= 2 KV heads),
computes its slice of Q/K/V projections, RoPE, causal attention, and a
partial output projection (rows g*512:(g+1)*512 of Wo). Host sums the 4
partials per batch.

Layout: projections are computed transposed (W^T-stationary matmuls on
x^T) producing Q^T/K^T ([feature, token]), the layout the scores matmul
needs. V is projected directly in [token, feature] layout (x^T tile as
the stationary operand) so no PE transposes are needed. Scores are
computed transposed (S^T[j,i], keys on partitions) so the context
matmul consumes exp(S^T) with V as the stationary operand; a ones
column appended to V yields softmax denominators in the same PSUM
accumulation.

Scheduling: attention is software-pipelined per head with a one-j-step
lag between the scores matmul and the context matmul that consumes its
exp, so the in-order PE never stalls waiting on the scalar engine.
"""

import sys
import math

for _p in ("/opt/trn_rl_repo",):
    if _p not in sys.path:
        sys.path.append(_p)

import numpy as np
import ml_dtypes

import concourse.bass as bass
from concourse import bacc
import concourse.mybir as mybir
import concourse.tile as tile
from concourse.bass_utils import run_bass_kernel_spmd

BF16 = mybir.dt.bfloat16
F32 = mybir.dt.float32

B, S, D = 2, 2048, 2048
NH, NKV, HD = 32, 8, 64
GROUP = NH // NKV          # 4 q heads per kv head
NCORES = 8
CPB = NCORES // B          # 4 cores per batch
HPC = NH // CPB            # 8 q heads per core
KVPC = NKV // CPB          # 2 kv heads per core
QW = HPC * HD              # 512 projected q cols per core
KW = KVPC * HD             # 128 projected kv cols per core

NT = S // 128              # 16 seq tiles of 128
NBL = S // 512             # 4 seq blocks of 512
KT = D // 128              # 16 contraction tiles
QF = QW // 128             # 4 row-tiles of Q^T

_nc_cache = None
_DEBUG = False


def _build():
    nc = bacc.Bacc()
    xT = nc.dram_tensor("xT", [D, S], BF16, kind="ExternalInput")
    wq = nc.dram_tensor("wq", [D, QW], BF16, kind="ExternalInput")
    wk = nc.dram_tensor("wk", [D, KW], BF16, kind="ExternalInput")
    wv = nc.dram_tensor("wv", [D, KW], BF16, kind="ExternalInput")
    wo = nc.dram_tensor("wo", [QW, D], BF16, kind="ExternalInput")
    cos2 = nc.dram_tensor("cos2", [128, S], F32, kind="ExternalInput")
    sinm = nc.dram_tensor("sinm", [128, S], F32, kind="ExternalInput")
    msk = nc.dram_tensor("msk", [128, 128], BF16, kind="ExternalInput")
    out = nc.dram_tensor("out", [S, D], BF16, kind="ExternalOutput")
    if _DEBUG:
        qdbg = nc.dram_tensor("qdbg", [QF, 128, S], BF16, kind="ExternalOutput")
        kdbg = nc.dram_tensor("kdbg", [128, S], BF16, kind="ExternalOutput")
        vdbg = nc.dram_tensor("vdbg", [KVPC, 128, NT * 65], BF16, kind="ExternalOutput")
        cdbg = nc.dram_tensor("cdbg", [QF, 128, S], BF16, kind="ExternalOutput")


    Exp = mybir.ActivationFunctionType.Exp

    with tile.TileContext(nc) as tc:
        with (
            tc.tile_pool(name="persist", bufs=1) as pp,
            tc.tile_pool(name="psum", bufs=1, space="PSUM") as psp,
        ):
            # ---- persistent tiles ----
            Qb = [pp.tile([128, S], BF16, name=f"qb{f}", tag=f"qb{f}") for f in range(QF)]
            Kb = pp.tile([128, S], BF16, name="kb", tag="kb")
            Vaug = [pp.tile([128, NT, 65], BF16, name=f"vaug{k}", tag=f"vaug{k}") for k in range(KVPC)]
            ctxT = [pp.tile([128, S], BF16, name=f"ctxt{f}", tag=f"ctxt{f}") for f in range(QF)]
            trit = pp.tile([128, 128], BF16, name="trit", tag="trit")

            for k in range(KVPC):
                nc.gpsimd.memset(Vaug[k][:, :, 64:65], 1.0)

            def st_tile():
                # slot sized for the attention [128,1024] scores tile; other
                # phases allocate [128,512]/[128,128] views from the same tag
                return psp.tile([128, 512], F32, name="st", tag="st",
                                bufs=2, padded_shape=[128, 1024])

            def st2_tile():
                return psp.tile([128, 1024], F32, name="st2", tag="st",
                                bufs=2)

            def cp_tile(i):
                return psp.tile([128, 1024], F32, name=f"cp{i}",
                                tag=f"cp{i % 2}", bufs=1)

            # ==== phase 1+2: load x/weights, projections + rope ====
            with (
                tc.tile_pool(name="proj", bufs=1) as jp,
                tc.tile_pool(name="rope", bufs=3) as rp,
            ):
                xt = [jp.tile([128, S], BF16, name=f"xt{k}", tag=f"xt{k}") for k in range(KT)]
                wqt = [jp.tile([128, QW], BF16, name=f"wqt{k}", tag=f"wqt{k}") for k in range(KT)]
                wkt = [jp.tile([128, KW], BF16, name=f"wkt{k}", tag=f"wkt{k}") for k in range(KT)]
                wvt = [jp.tile([128, KW], BF16, name=f"wvt{k}", tag=f"wvt{k}") for k in range(KT)]
                cos2t = jp.tile([128, S], F32, name="cos2t", tag="cos2t")
                sinmt = jp.tile([128, S], F32, name="sinmt", tag="sinmt")

                # DMA issue order chosen so compute can start early: wk +
                # first x block unlock K t0; cos/sin unlock its rope; then
                # wq/wv unlock Q and V while the rest of x streams in.
                for k in range(KT):
                    nc.sync.dma_start(wkt[k][:], wk[k * 128:(k + 1) * 128, :])
                for k in range(KT):
                    nc.sync.dma_start(xt[k][:, 0:512], xT[k * 128:(k + 1) * 128, 0:512])
                nc.sync.dma_start(cos2t[:], cos2[:, :])
                nc.sync.dma_start(sinmt[:], sinm[:, :])
                for k in range(KT):
                    nc.sync.dma_start(wqt[k][:], wq[k * 128:(k + 1) * 128, :])
                for k in range(KT):
                    nc.sync.dma_start(wvt[k][:], wv[k * 128:(k + 1) * 128, :])
                for k in range(KT):
                    nc.sync.dma_start(xt[k][:, 512:S], xT[k * 128:(k + 1) * 128, 512:S])
                nc.sync.dma_start(trit[:], msk[:, :])

                def rope_store(ps, dst, tcol):
                    # ps: psum [128, 512] f32 holding raw Q^T/K^T rows.
                    # dst[:, tcol:tcol+512] <- rope(ps) in bf16.
                    qf = rp.tile([128, 512], F32, name="ropecp", tag="ropecp")
                    nc.scalar.copy(qf[:], ps[:])
                    rot = rp.tile([128, 512], F32, name="roperot", tag="roperot")
                    for base in (0, 64):
                        nc.sync.dma_start(rot[base:base + 32, :],
                                          qf[base + 32:base + 64, :])
                        nc.sync.dma_start(rot[base + 32:base + 64, :],
                                          qf[base:base + 32, :])
                    a = rp.tile([128, 512], F32, name="ropea", tag="ropea")
                    b = rp.tile([128, 512], F32, name="ropeb", tag="ropeb")
                    nc.vector.tensor_mul(a[:], qf[:], cos2t[:, tcol:tcol + 512])
                    nc.vector.tensor_mul(b[:], rot[:], sinmt[:, tcol:tcol + 512])
                    nc.vector.tensor_add(dst[:, tcol:tcol + 512], a[:], b[:])

                def proj_k(t):
                    ps = st_tile()
                    for k in range(KT):
                        nc.tensor.matmul(
                            ps[:], wkt[k][:], xt[k][:, t * 512:(t + 1) * 512],
                            start=(k == 0), stop=(k == KT - 1))
                    rope_store(ps, Kb, t * 512)

                def proj_q(f, t):
                    ps = st_tile()
                    for k in range(KT):
                        nc.tensor.matmul(
                            ps[:], wqt[k][:, f * 128:(f + 1) * 128],
                            xt[k][:, t * 512:(t + 1) * 512],
                            start=(k == 0), stop=(k == KT - 1))
                    rope_store(ps, Qb[f], t * 512)

                def proj_v(tt):
                    # V directly in [token, feature] layout: x^T tile is the
                    # stationary operand, wv streams. out [128 tok, 128 feat].
                    ps = psp.tile([128, 128], F32, name="vp", tag="st", bufs=2,
                                   padded_shape=[128, 1024])
                    for k in range(KT):
                        nc.tensor.matmul(
                            ps[:], xt[k][:, tt * 128:(tt + 1) * 128],
                            wvt[k][:, 0:KW],
                            start=(k == 0), stop=(k == KT - 1))
                    for kv in range(KVPC):
                        nc.scalar.copy(Vaug[kv][:, tt, 0:64],
                                       ps[:, kv * 64:(kv + 1) * 64])

                for t in range(NBL):
                    proj_k(t)
                    for f in range(QF):
                        proj_q(f, t)
                    for tt in range(4 * t, 4 * t + 4):
                        proj_v(tt)

            # ==== phase 3: attention per local q head ====
            with (
                tc.tile_pool(name="attn", bufs=8) as ap,
                tc.tile_pool(name="smal", bufs=2) as sp,
                tc.tile_pool(name="wout", bufs=1) as wp,
                tc.tile_pool(name="ostg", bufs=2) as op,
            ):
                wot = [wp.tile([128, D], BF16, name=f"wot{c}", tag=f"wot{c}") for c in range(QF)]
                for c in range(QF):
                    nc.sync.dma_start(wot[c][:], wo[c * 128:(c + 1) * 128, :])

                # Q tiles are head-permuted (host): tile f holds local
                # heads f (kv0, rows 0:64) and f+4 (kv1, rows 64:128), so
                # the K lhsT base partition always matches the Q rhs base.
                IB2 = 2          # 1024-wide query blocks
                for hl in range(HPC):
                    kv = hl // GROUP
                    fq = hl % GROUP
                    qr = kv * 64
                    cp = [cp_tile(i) for i in range(IB2)]
                    prev = []

                    def do_scores(j):
                        cur = []
                        jb = j * 128
                        for ib in range(j // 8, IB2):
                            q0 = ib * 1024
                            c0 = max(0, jb - q0)
                            st = st2_tile()
                            for lo in (0, 512):
                                hi = lo + 512
                                if hi <= c0:
                                    continue
                                l0 = max(lo, c0)
                                nc.tensor.matmul(
                                    st[:, l0:hi],
                                    Kb[kv * 64:(kv + 1) * 64,
                                       jb:jb + 128],
                                    Qb[fq][qr:qr + 64, q0 + l0:q0 + hi],
                                    start=True, stop=True)
                            pt = ap.tile([128, 1024], BF16, name="pt", tag="pt")
                            nc.scalar.activation(pt[:, c0:1024], st[:, c0:1024],
                                                 Exp, scale=0.125)
                            if 0 <= jb - q0 < 1024:
                                # diagonal tile: only the 128-wide band at the
                                # causal boundary needs the triangle mask
                                nc.vector.tensor_mul(pt[:, c0:c0 + 128],
                                                     pt[:, c0:c0 + 128],
                                                     trit[:])
                            cur.append((j, ib, c0, pt))
                        return cur

                    def do_ctx(items):
                        for (j, ib, c0, pt) in items:
                            for lo in (0, 512):
                                hi = lo + 512
                                if hi <= c0:
                                    continue
                                l0 = max(lo, c0)
                                nc.tensor.matmul(
                                    cp[ib][0:65, l0:hi], Vaug[kv][:, j, :],
                                    pt[:, l0:hi],
                                    start=(j == 0), stop=(j == 8 * ib + 7),
                                    skip_group_check=True)
                            if j == 8 * ib + 7:
                                norm_ctx(ib)

                    def norm_ctx(ib):
                        # denominators sit in row 64 of cp[ib]; normalize the
                        # 64 ctx rows and store into ctxT, freeing the bank.
                        # (denom must bounce via SBUF: custom-DVE ops read
                        # garbage from PSUM on hw)
                        dn = sp.tile([1, 1024], F32, name="dn", tag="dn")
                        nc.vector.tensor_copy(dn[0:1, :], cp[ib][64:65, :])
                        rc = sp.tile([1, 1024], F32, name="rc", tag="rc")
                        nc.vector.reciprocal_approx_fast(rc[0:1, :], dn[0:1, :])
                        bc = sp.tile([64, 1024], F32, name="bc", tag="bc")
                        nc.gpsimd.partition_broadcast(bc[0:64, :], rc[0:1, :])
                        nc.vector.tensor_mul(
                            ctxT[fq][qr:qr + 64, ib * 1024:(ib + 1) * 1024],
                            cp[ib][0:64, :], bc[0:64, :])

                    for j in range(NT):
                        cur = do_scores(j)
                        do_ctx(prev)
                        prev = cur
                    do_ctx(prev)

                # ==== phase 4: partial out = ctx @ Wo_slice ====
                if _DEBUG:
                    for f in range(QF):
                        nc.sync.dma_start(qdbg[f, :, :], Qb[f][:])
                        nc.sync.dma_start(cdbg[f, :, :], ctxT[f][:])
                    nc.sync.dma_start(kdbg[:, :], Kb[:])
                    for kv in range(KVPC):
                        nc.sync.dma_start(vdbg[kv, :, :],
                                          Vaug[kv][:, :, :])
                    pass

                for t in range(NT):
                    ob = op.tile([128, D], BF16, name="ob", tag="ob")
                    for o in range(NBL):
                        ps = st_tile()
                        for c in range(QF):
                            nc.tensor.matmul(
                                ps[:],
                                ctxT[c][:, t * 128:(t + 1) * 128],
                                wot[c][:, o * 512:(o + 1) * 512],
                                start=(c == 0), stop=(c == QF - 1))
                        nc.vector.tensor_copy(ob[:, o * 512:(o + 1) * 512],
                                              ps[:])
                    nc.sync.dma_start(out[t * 128:(t + 1) * 128, :], ob[:])

    nc.finalize()
    return nc


def _get_nc():
    global _nc_cache
    if _nc_cache is None:
        _nc_cache = _build()
    return _nc_cache


def _prep_inputs(x, cos, sin, Wq, Wk, Wv, Wo):
    bf = ml_dtypes.bfloat16
    cosT = np.ascontiguousarray(cos.T.astype(np.float32))          # [64, S]
    sinT = sin.T.astype(np.float32)
    sinm64 = np.concatenate([-sinT[:32], sinT[32:]], axis=0)       # [64, S]
    cos2 = np.ascontiguousarray(np.concatenate([cosT, cosT], 0))   # [128, S]
    sinm = np.ascontiguousarray(np.concatenate([sinm64, sinm64], 0))
    msk = (np.arange(128)[:, None] <= np.arange(128)[None, :]).astype(bf)

    # head permutation: Q^T tile f holds local heads (f, f+4) so that the
    # kv0/kv1 row base of K matches the q row base (PE base-partition rule)
    perm = [0, 4, 1, 5, 2, 6, 3, 7]
    colperm = np.concatenate(
        [np.arange(HD) + p * HD for p in perm])          # [QW]
    in_maps = []
    for c in range(NCORES):
        b, g = c // CPB, c % CPB
        xTb = np.ascontiguousarray(x[b].T.astype(bf))
        wq_g = Wq[:, g * QW:(g + 1) * QW][:, colperm]
        wo_g = Wo[g * QW:(g + 1) * QW, :][colperm, :]
        in_maps.append({
            "xT": xTb,
            "wq": np.ascontiguousarray(wq_g.astype(bf)),
            "wk": np.ascontiguousarray(Wk[:, g * KW:(g + 1) * KW].astype(bf)),
            "wv": np.ascontiguousarray(Wv[:, g * KW:(g + 1) * KW].astype(bf)),
            "wo": np.ascontiguousarray(wo_g.astype(bf)),
            "cos2": cos2,
            "sinm": sinm,
            "msk": msk,
        })
    return in_maps


def kernel(x, mask, cos, sin, Wq, Wk, Wv, Wo, _trace=False, **kw):
    x = np.asarray(x, dtype=np.float32)
    in_maps = _prep_inputs(x, np.asarray(cos), np.asarray(sin),
                           np.asarray(Wq), np.asarray(Wk),
                           np.asarray(Wv), np.asarray(Wo))
    nc = _get_nc()
    res = run_bass_kernel_spmd(nc, in_maps, core_ids=list(range(NCORES)),
                               trace=_trace, **kw)
    parts = [np.asarray(r["out"], dtype=np.float32) for r in res.results]
    full = np.stack([
        sum(parts[b * CPB + g] for g in range(CPB)) for b in range(B)
    ]).astype(np.float32)
    if _trace:
        kernel.last_result = res
    return full


# revision 14
# speedup vs baseline: 1.3261x; 1.1365x over previous
"""GQA attention kernel for Trainium2, 8 NeuronCores.

Sharding: core c -> (batch = c // 4, head-group g = c % 4).
Each core handles one batch and 8 contiguous Q heads (= 2 KV heads),
computes its slice of Q/K/V projections, RoPE, causal attention, and a
partial output projection (rows g*512:(g+1)*512 of Wo). Host sums the 4
partials per batch.

Layout: projections are computed transposed (W^T-stationary matmuls on
x^T) producing Q^T/K^T ([feature, token]), the layout the scores matmul
needs. V is projected directly in [token, feature] layout (x^T tile as
the stationary operand) so no PE transposes are needed. Scores are
computed transposed (S^T[j,i], keys on partitions) so the context
matmul consumes exp(S^T) with V as the stationary operand; a ones
column appended to V yields softmax denominators in the same PSUM
accumulation.

Scheduling: attention is software-pipelined per head with a one-j-step
lag between the scores matmul and the context matmul that consumes its
exp, so the in-order PE never stalls waiting on the scalar engine.
"""

import sys
import math

for _p in ("/opt/trn_rl_repo",):
    if _p not in sys.path:
        sys.path.append(_p)

import numpy as np
import ml_dtypes

import concourse.bass as bass
from concourse import bacc
import concourse.mybir as mybir
import concourse.tile as tile
from concourse.bass_utils import run_bass_kernel_spmd

BF16 = mybir.dt.bfloat16
F32 = mybir.dt.float32

B, S, D = 2, 2048, 2048
NH, NKV, HD = 32, 8, 64
GROUP = NH // NKV          # 4 q heads per kv head
NCORES = 8
CPB = NCORES // B          # 4 cores per batch
HPC = NH // CPB            # 8 q heads per core
KVPC = NKV // CPB          # 2 kv heads per core
QW = HPC * HD              # 512 projected q cols per core
KW = KVPC * HD             # 128 projected kv cols per core

NT = S // 128              # 16 seq tiles of 128
NBL = S // 512             # 4 seq blocks of 512
KT = D // 128              # 16 contraction tiles
QF = QW // 128             # 4 row-tiles of Q^T

_nc_cache = None
_DEBUG = False


def _build():
    nc = bacc.Bacc()
    xT = nc.dram_tensor("xT", [D, S], BF16, kind="ExternalInput")
    wq = nc.dram_tensor("wq", [D, QW], BF16, kind="ExternalInput")
    wk = nc.dram_tensor("wk", [D, KW], BF16, kind="ExternalInput")
    wv = nc.dram_tensor("wv", [D, KW], BF16, kind="ExternalInput")
    wo = nc.dram_tensor("wo", [QW, D], BF16, kind="ExternalInput")
    cos2 = nc.dram_tensor("cos2", [128, S], F32, kind="ExternalInput")
    sinm = nc.dram_tensor("sinm", [128, S], F32, kind="ExternalInput")
    msk = nc.dram_tensor("msk", [128, 128], BF16, kind="ExternalInput")
    out = nc.dram_tensor("out", [S, D], BF16, kind="ExternalOutput")
    if _DEBUG:
        qdbg = nc.dram_tensor("qdbg", [QF, 128, S], BF16, kind="ExternalOutput")
        kdbg = nc.dram_tensor("kdbg", [128, S], BF16, kind="ExternalOutput")
        vdbg = nc.dram_tensor("vdbg", [KVPC, 128, NT * 65], BF16, kind="ExternalOutput")
        cdbg = nc.dram_tensor("cdbg", [QF, 128, S], BF16, kind="ExternalOutput")


    Exp = mybir.ActivationFunctionType.Exp

    with tile.TileContext(nc) as tc:
        with (
            tc.tile_pool(name="persist", bufs=1) as pp,
            tc.tile_pool(name="psum", bufs=1, space="PSUM") as psp,
        ):
            # ---- persistent tiles ----
            Qb = [pp.tile([128, S], BF16, name=f"qb{f}", tag=f"qb{f}") for f in range(QF)]
            Kb = pp.tile([128, S], BF16, name="kb", tag="kb")
            Vaug = [pp.tile([128, NT, 65], BF16, name=f"vaug{k}", tag=f"vaug{k}") for k in range(KVPC)]
            ctxT = [pp.tile([128, S], BF16, name=f"ctxt{f}", tag=f"ctxt{f}") for f in range(QF)]
            trit = pp.tile([128, 128], BF16, name="trit", tag="trit")

            for k in range(KVPC):
                nc.gpsimd.memset(Vaug[k][:, :, 64:65], 1.0)

            def st_tile():
                # slot sized for the attention [128,1024] scores tile; other
                # phases allocate [128,512]/[128,128] views from the same tag
                return psp.tile([128, 512], F32, name="st", tag="st",
                                bufs=2, padded_shape=[128, 1024])

            def st2_tile():
                return psp.tile([128, 1024], F32, name="st2", tag="st",
                                bufs=2)

            def cp_tile(i):
                return psp.tile([128, 1024], F32, name=f"cp{i}",
                                tag=f"cp{i % 2}", bufs=1)

            # ==== phase 1+2: load x/weights, projections + rope ====
            with (
                tc.tile_pool(name="proj", bufs=1) as jp,
                tc.tile_pool(name="rope", bufs=3) as rp,
            ):
                xt = [jp.tile([128, S], BF16, name=f"xt{k}", tag=f"xt{k}") for k in range(KT)]
                wqt = [jp.tile([128, QW], BF16, name=f"wqt{k}", tag=f"wqt{k}") for k in range(KT)]
                wkt = [jp.tile([128, KW], BF16, name=f"wkt{k}", tag=f"wkt{k}") for k in range(KT)]
                wvt = [jp.tile([128, KW], BF16, name=f"wvt{k}", tag=f"wvt{k}") for k in range(KT)]
                cos2t = jp.tile([128, S], F32, name="cos2t", tag="cos2t")
                sinmt = jp.tile([128, S], F32, name="sinmt", tag="sinmt")

                # DMA issue order matches the compute schedule below so the
                # PE never waits mid-phase: K t0 and V t0 unlock first.
                for k in range(KT):
                    nc.sync.dma_start(wkt[k][:], wk[k * 128:(k + 1) * 128, :])
                for k in range(KT):
                    nc.sync.dma_start(wvt[k][:], wv[k * 128:(k + 1) * 128, :])
                for k in range(KT):
                    nc.sync.dma_start(xt[k][:, 0:512], xT[k * 128:(k + 1) * 128, 0:512])
                nc.sync.dma_start(cos2t[:], cos2[:, :])
                nc.sync.dma_start(sinmt[:], sinm[:, :])
                for k in range(KT):
                    nc.sync.dma_start(xt[k][:, 512:1024], xT[k * 128:(k + 1) * 128, 512:1024])
                for k in range(KT):
                    nc.sync.dma_start(wqt[k][:], wq[k * 128:(k + 1) * 128, :])
                for k in range(KT):
                    nc.sync.dma_start(xt[k][:, 1024:1536], xT[k * 128:(k + 1) * 128, 1024:1536])
                for k in range(KT):
                    nc.sync.dma_start(xt[k][:, 1536:S], xT[k * 128:(k + 1) * 128, 1536:S])
                nc.sync.dma_start(trit[:], msk[:, :])

                def rope_store(ps, dst, tcol):
                    # ps: psum [128, 512] f32 holding raw Q^T/K^T rows.
                    # dst[:, tcol:tcol+512] <- rope(ps) in bf16.
                    qf = rp.tile([128, 512], F32, name="ropecp", tag="ropecp")
                    nc.scalar.copy(qf[:], ps[:])
                    rot = rp.tile([128, 512], F32, name="roperot", tag="roperot")
                    for base in (0, 64):
                        nc.sync.dma_start(rot[base:base + 32, :],
                                          qf[base + 32:base + 64, :])
                        nc.sync.dma_start(rot[base + 32:base + 64, :],
                                          qf[base:base + 32, :])
                    a = rp.tile([128, 512], F32, name="ropea", tag="ropea")
                    b = rp.tile([128, 512], F32, name="ropeb", tag="ropeb")
                    nc.vector.tensor_mul(a[:], qf[:], cos2t[:, tcol:tcol + 512])
                    nc.vector.tensor_mul(b[:], rot[:], sinmt[:, tcol:tcol + 512])
                    nc.vector.tensor_add(dst[:, tcol:tcol + 512], a[:], b[:])

                def proj_k(t):
                    ps = st_tile()
                    for k in range(KT):
                        nc.tensor.matmul(
                            ps[:], wkt[k][:], xt[k][:, t * 512:(t + 1) * 512],
                            start=(k == 0), stop=(k == KT - 1))
                    rope_store(ps, Kb, t * 512)

                def proj_q(f, t):
                    ps = st_tile()
                    for k in range(KT):
                        nc.tensor.matmul(
                            ps[:], wqt[k][:, f * 128:(f + 1) * 128],
                            xt[k][:, t * 512:(t + 1) * 512],
                            start=(k == 0), stop=(k == KT - 1))
                    rope_store(ps, Qb[f], t * 512)

                def proj_v(tt):
                    # V directly in [token, feature] layout: x^T tile is the
                    # stationary operand, wv streams. out [128 tok, 128 feat].
                    ps = psp.tile([128, 128], F32, name="vp", tag="st", bufs=2,
                                   padded_shape=[128, 1024])
                    for k in range(KT):
                        nc.tensor.matmul(
                            ps[:], xt[k][:, tt * 128:(tt + 1) * 128],
                            wvt[k][:, 0:KW],
                            start=(k == 0), stop=(k == KT - 1))
                    for kv in range(KVPC):
                        nc.vector.tensor_copy(Vaug[kv][:, tt, 0:64],
                                              ps[:, kv * 64:(kv + 1) * 64])

                proj_k(0)
                for tt in range(0, 4):
                    proj_v(tt)
                proj_k(1)
                for tt in range(4, 8):
                    proj_v(tt)
                for f in range(QF):
                    proj_q(f, 0)
                proj_k(2)
                for tt in range(8, 12):
                    proj_v(tt)
                for f in range(QF):
                    proj_q(f, 1)
                proj_k(3)
                for tt in range(12, 16):
                    proj_v(tt)
                for f in range(QF):
                    proj_q(f, 2)
                for f in range(QF):
                    proj_q(f, 3)

            # ==== phase 3: attention per local q head ====
            with (
                tc.tile_pool(name="attn", bufs=8) as ap,
                tc.tile_pool(name="smal", bufs=2) as sp,
                tc.tile_pool(name="wout", bufs=1) as wp,
                tc.tile_pool(name="ostg", bufs=2) as op,
            ):
                wot = [wp.tile([128, D], BF16, name=f"wot{c}", tag=f"wot{c}") for c in range(QF)]
                for c in range(QF):
                    nc.sync.dma_start(wot[c][:], wo[c * 128:(c + 1) * 128, :])

                # Q tiles are head-permuted (host): tile f holds local
                # heads f (kv0, rows 0:64) and f+4 (kv1, rows 64:128), so
                # the K lhsT base partition always matches the Q rhs base.
                IB2 = 2          # 1024-wide query blocks

                def outproj_tile(t, tagid):
                    # one full output token-tile: 16 dense matmuls with no
                    # cross-engine deps — a long wait-free PE run that fires
                    # the HAM warm-up when injected inside attention.
                    ob = op.tile([128, D], BF16, name="ob", tag="ob")
                    ps2 = psp.tile([128, 2, 512], F32, name="ops",
                                   tag=f"cp{tagid}", bufs=1)
                    for o in range(NBL):
                        h = o % 2
                        for c in range(QF):
                            nc.tensor.matmul(
                                ps2[:, h, :],
                                ctxT[c][:, t * 128:(t + 1) * 128],
                                wot[c][:, o * 512:(o + 1) * 512],
                                start=(c == 0), stop=(c == QF - 1))
                        nc.vector.tensor_copy(ob[:, o * 512:(o + 1) * 512],
                                              ps2[:, h, :])
                    nc.sync.dma_start(out[t * 128:(t + 1) * 128, :], ob[:])

                def attn_block(hl, ib2, inject=None):
                    kv = hl // GROUP
                    fq = hl % GROUP
                    qr = kv * 64
                    q0 = ib2 * 1024
                    jmax = 8 * ib2 + 7
                    cp = cp_tile(hl % 2)

                    def do_scores(j):
                        jb = j * 128
                        c0 = max(0, jb - q0)
                        st = st2_tile()
                        for lo in (0, 512):
                            hi = lo + 512
                            if hi <= c0:
                                continue
                            l0 = max(lo, c0)
                            nc.tensor.matmul(
                                st[:, l0:hi],
                                Kb[kv * 64:(kv + 1) * 64, jb:jb + 128],
                                Qb[fq][qr:qr + 64, q0 + l0:q0 + hi],
                                start=True, stop=True)
                        pt = ap.tile([128, 1024], BF16, name="pt", tag="pt")
                        nc.scalar.activation(pt[:, c0:1024], st[:, c0:1024],
                                             Exp, scale=0.125)
                        if 0 <= jb - q0 < 1024:
                            # diagonal tile: only the 128-wide band at the
                            # causal boundary needs the triangle mask
                            nc.vector.tensor_mul(pt[:, c0:c0 + 128],
                                                 pt[:, c0:c0 + 128],
                                                 trit[:])
                        return (j, c0, pt)

                    def do_ctx(item):
                        (j, c0, pt) = item
                        for lo in (0, 512):
                            hi = lo + 512
                            if hi <= c0:
                                continue
                            l0 = max(lo, c0)
                            nc.tensor.matmul(
                                cp[0:65, l0:hi], Vaug[kv][:, j, :],
                                pt[:, l0:hi],
                                start=(j == 0), stop=(j == jmax),
                                skip_group_check=True)
                        if j == jmax:
                            norm_ctx()

                    def norm_ctx():
                        # denominators sit in row 64 of cp; normalize the
                        # 64 ctx rows and store into ctxT, freeing the bank.
                        # (denom must bounce via SBUF: custom-DVE ops read
                        # garbage from PSUM on hw)
                        dn = sp.tile([1, 1024], F32, name="dn", tag="dn")
                        nc.vector.tensor_copy(dn[0:1, :], cp[64:65, :])
                        rc = sp.tile([1, 1024], F32, name="rc", tag="rc")
                        nc.vector.reciprocal_approx_fast(rc[0:1, :], dn[0:1, :])
                        bc = sp.tile([64, 1024], F32, name="bc", tag="bc")
                        nc.gpsimd.partition_broadcast(bc[0:64, :], rc[0:1, :])
                        nc.vector.tensor_mul(
                            ctxT[fq][qr:qr + 64, q0:q0 + 1024],
                            cp[0:64, :], bc[0:64, :])

                    prev = None
                    for j in range(jmax + 1):
                        cur = do_scores(j)
                        if prev is not None:
                            do_ctx(prev)
                        if inject is not None and j == 8:
                            inject()
                        prev = cur
                    do_ctx(prev)

                # phase 3a: first 1024 queries, all heads
                for hl in range(HPC):
                    attn_block(hl, 0)
                # phase 3b: second 1024 queries; inject an output-projection
                # token-tile (for the already-finished first half) per head
                for hl in range(HPC):
                    attn_block(hl, 1,
                               inject=(lambda t=hl, g=(hl + 1) % 2:
                                       outproj_tile(t, g)))

                # ==== phase 4: remaining out token-tiles ====
                for t in range(8, NT):
                    outproj_tile(t, t % 2)

                if _DEBUG:
                    for f in range(QF):
                        nc.sync.dma_start(qdbg[f, :, :], Qb[f][:])
                        nc.sync.dma_start(cdbg[f, :, :], ctxT[f][:])
                    nc.sync.dma_start(kdbg[:, :], Kb[:])
                    for kv in range(KVPC):
                        nc.sync.dma_start(vdbg[kv, :, :],
                                          Vaug[kv][:, :, :])
                    pass


    nc.finalize()
    return nc


def _get_nc():
    global _nc_cache
    if _nc_cache is None:
        _nc_cache = _build()
    return _nc_cache


def _prep_inputs(x, cos, sin, Wq, Wk, Wv, Wo):
    bf = ml_dtypes.bfloat16
    cosT = np.ascontiguousarray(cos.T.astype(np.float32))          # [64, S]
    sinT = sin.T.astype(np.float32)
    sinm64 = np.concatenate([-sinT[:32], sinT[32:]], axis=0)       # [64, S]
    cos2 = np.ascontiguousarray(np.concatenate([cosT, cosT], 0))   # [128, S]
    sinm = np.ascontiguousarray(np.concatenate([sinm64, sinm64], 0))
    msk = (np.arange(128)[:, None] <= np.arange(128)[None, :]).astype(bf)

    # head permutation: Q^T tile f holds local heads (f, f+4) so that the
    # kv0/kv1 row base of K matches the q row base (PE base-partition rule)
    perm = [0, 4, 1, 5, 2, 6, 3, 7]
    colperm = np.concatenate(
        [np.arange(HD) + p * HD for p in perm])          # [QW]
    in_maps = []
    for c in range(NCORES):
        b, g = c // CPB, c % CPB
        xTb = np.ascontiguousarray(x[b].T.astype(bf))
        wq_g = Wq[:, g * QW:(g + 1) * QW][:, colperm]
        wo_g = Wo[g * QW:(g + 1) * QW, :][colperm, :]
        in_maps.append({
            "xT": xTb,
            "wq": np.ascontiguousarray(wq_g.astype(bf)),
            "wk": np.ascontiguousarray(Wk[:, g * KW:(g + 1) * KW].astype(bf)),
            "wv": np.ascontiguousarray(Wv[:, g * KW:(g + 1) * KW].astype(bf)),
            "wo": np.ascontiguousarray(wo_g.astype(bf)),
            "cos2": cos2,
            "sinm": sinm,
            "msk": msk,
        })
    return in_maps


def kernel(x, mask, cos, sin, Wq, Wk, Wv, Wo, _trace=False, **kw):
    x = np.asarray(x, dtype=np.float32)
    in_maps = _prep_inputs(x, np.asarray(cos), np.asarray(sin),
                           np.asarray(Wq), np.asarray(Wk),
                           np.asarray(Wv), np.asarray(Wo))
    nc = _get_nc()
    res = run_bass_kernel_spmd(nc, in_maps, core_ids=list(range(NCORES)),
                               trace=_trace, **kw)
    parts = [np.asarray(r["out"], dtype=np.float32) for r in res.results]
    full = np.stack([
        sum(parts[b * CPB + g] for g in range(CPB)) for b in range(B)
    ]).astype(np.float32)
    if _trace:
        kernel.last_result = res
    return full


# revision 15
# speedup vs baseline: 1.4596x; 1.1007x over previous
"""GQA attention kernel for Trainium2, 8 NeuronCores.

Sharding: core c -> (batch = c // 4, head-group g = c % 4).
Each core handles one batch and 8 contiguous Q heads (= 2 KV heads),
computes its slice of Q/K/V projections, RoPE, causal attention, and a
partial output projection (rows g*512:(g+1)*512 of Wo). Host sums the 4
partials per batch.

Layout: projections are computed transposed (W^T-stationary matmuls on
x^T) producing Q^T/K^T ([feature, token]), the layout the scores matmul
needs. V is projected directly in [token, feature] layout (x^T tile as
the stationary operand) so no PE transposes are needed. Scores are
computed transposed (S^T[j,i], keys on partitions) in 1024-query
blocks so the context matmul consumes exp(S^T) with V as the stationary
operand; a ones column appended to V yields softmax denominators in the
same PSUM accumulation.

Scheduling is built around the PE HAM clock gate (PE runs at 1.2 GHz
until it sees a ~3.4us window of gapless activity, then 2.4 GHz):
 - attention is software-pipelined per head with a one-j-step lag
   between scores and the context matmul consuming its exp;
 - the projection tail is interleaved between attention heads of the
   first query half, and output-projection token-tiles (16 dense
   back-to-back matmuls each) are injected between heads of the second
   half, so the PE always has long wait-free runs that keep the clock
   warm;
 - bulk inputs are loaded with single rearranged DMAs (dma_start issue
   costs ~0.6us each on the sync queue), and small rope/output DMAs go
   through the gpsimd queue.
"""

import sys
import math

for _p in ("/opt/trn_rl_repo",):
    if _p not in sys.path:
        sys.path.append(_p)

import numpy as np
import ml_dtypes

import concourse.bass as bass
from concourse import bacc
import concourse.mybir as mybir
import concourse.tile as tile
from concourse.bass_utils import run_bass_kernel_spmd

BF16 = mybir.dt.bfloat16
F32 = mybir.dt.float32

B, S, D = 2, 2048, 2048
NH, NKV, HD = 32, 8, 64
GROUP = NH // NKV          # 4 q heads per kv head
NCORES = 8
CPB = NCORES // B          # 4 cores per batch
HPC = NH // CPB            # 8 q heads per core
KVPC = NKV // CPB          # 2 kv heads per core
QW = HPC * HD              # 512 projected q cols per core
KW = KVPC * HD             # 128 projected kv cols per core

NT = S // 128              # 16 seq tiles of 128
NBL = S // 512             # 4 seq blocks of 512
KT = D // 128              # 16 contraction tiles
QF = QW // 128             # 4 row-tiles of Q^T

_nc_cache = None
_DEBUG = False


def _build():
    nc = bacc.Bacc()
    xT = nc.dram_tensor("xT", [D, S], BF16, kind="ExternalInput")
    wq = nc.dram_tensor("wq", [D, QW], BF16, kind="ExternalInput")
    wk = nc.dram_tensor("wk", [D, KW], BF16, kind="ExternalInput")
    wv = nc.dram_tensor("wv", [D, KW], BF16, kind="ExternalInput")
    wo = nc.dram_tensor("wo", [QW, D], BF16, kind="ExternalInput")
    cos2 = nc.dram_tensor("cos2", [128, S], BF16, kind="ExternalInput")
    sinm = nc.dram_tensor("sinm", [128, S], BF16, kind="ExternalInput")
    msk = nc.dram_tensor("msk", [128, 128], BF16, kind="ExternalInput")
    out = nc.dram_tensor("out", [S, D], BF16, kind="ExternalOutput")
    if _DEBUG:
        qdbg = nc.dram_tensor("qdbg", [QF, 128, S], BF16, kind="ExternalOutput")
        kdbg = nc.dram_tensor("kdbg", [128, S], BF16, kind="ExternalOutput")
        vdbg = nc.dram_tensor("vdbg", [KVPC, 128, NT * 65], BF16, kind="ExternalOutput")
        cdbg = nc.dram_tensor("cdbg", [QF, 128, S], BF16, kind="ExternalOutput")

    Exp = mybir.ActivationFunctionType.Exp

    with tile.TileContext(nc) as tc:
        with (
            tc.tile_pool(name="persist", bufs=1) as pp,
            tc.tile_pool(name="psum", bufs=1, space="PSUM") as psp,
            tc.tile_pool(name="attn", bufs=4) as ap,
            tc.tile_pool(name="smal", bufs=1) as sp,
            tc.tile_pool(name="ostg", bufs=2) as op,
        ):
            # ---- persistent tiles ----
            Qb = [pp.tile([128, S], BF16, name=f"qb{f}", tag=f"qb{f}") for f in range(QF)]
            Kb = pp.tile([128, S], BF16, name="kb", tag="kb")
            Vaug = [pp.tile([128, NT, 65], BF16, name=f"vaug{k}", tag=f"vaug{k}") for k in range(KVPC)]
            ctxT = [pp.tile([128, S], BF16, name=f"ctxt{f}", tag=f"ctxt{f}") for f in range(QF)]
            trit = pp.tile([128, 128], BF16, name="trit", tag="trit")

            for k in range(KVPC):
                nc.gpsimd.memset(Vaug[k][:, :, 64:65], 1.0)

            def st_tile():
                return psp.tile([128, 512], F32, name="st", tag="st",
                                bufs=2, padded_shape=[128, 1024])

            def st2_tile():
                return psp.tile([128, 1024], F32, name="st2", tag="st",
                                bufs=2)

            def cp_tile(i):
                return psp.tile([128, 1024], F32, name=f"cp{i}",
                                tag=f"cp{i % 2}", bufs=1)

            # ---- attention block (1024-query half ib2 of head hl) ----
            def attn_block(hl, ib2, inject=None):
                kv = hl // GROUP
                fq = hl % GROUP
                qr = kv * 64
                q0 = ib2 * 1024
                jmax = 8 * ib2 + 7
                cp = cp_tile(hl % 2)

                def do_scores(j):
                    jb = j * 128
                    c0 = max(0, jb - q0)
                    st = st2_tile()
                    for lo in (0, 512):
                        hi = lo + 512
                        if hi <= c0:
                            continue
                        l0 = max(lo, c0)
                        nc.tensor.matmul(
                            st[:, l0:hi],
                            Kb[kv * 64:(kv + 1) * 64, jb:jb + 128],
                            Qb[fq][qr:qr + 64, q0 + l0:q0 + hi],
                            start=True, stop=True)
                    pt = ap.tile([128, 1024], BF16, name="pt", tag="pt")
                    nc.scalar.activation(pt[:, c0:1024], st[:, c0:1024],
                                         Exp, scale=0.125)
                    if 0 <= jb - q0 < 1024:
                        # diagonal tile: only the 128-wide band at the
                        # causal boundary needs the triangle mask
                        nc.vector.tensor_mul(pt[:, c0:c0 + 128],
                                             pt[:, c0:c0 + 128],
                                             trit[:])
                    return (j, c0, pt)

                def do_ctx(item):
                    (j, c0, pt) = item
                    for lo in (0, 512):
                        hi = lo + 512
                        if hi <= c0:
                            continue
                        l0 = max(lo, c0)
                        nc.tensor.matmul(
                            cp[0:65, l0:hi], Vaug[kv][:, j, :],
                            pt[:, l0:hi],
                            start=(j == 0), stop=(j == jmax),
                            skip_group_check=True)
                    if j == jmax:
                        norm_ctx()

                def norm_ctx():
                    # denominators sit in row 64 of cp; normalize the 64 ctx
                    # rows into ctxT, freeing the bank. (denom must bounce
                    # via SBUF: custom-DVE ops read garbage from PSUM on hw)
                    dn = sp.tile([1, 1024], F32, name="dn", tag="dn")
                    nc.vector.tensor_copy(dn[0:1, :], cp[64:65, :])
                    rc = sp.tile([1, 1024], F32, name="rc", tag="rc")
                    nc.vector.reciprocal_approx_fast(rc[0:1, :], dn[0:1, :])
                    bc = sp.tile([64, 1024], F32, name="bc", tag="bc")
                    nc.gpsimd.partition_broadcast(bc[0:64, :], rc[0:1, :])
                    nc.vector.tensor_mul(
                        ctxT[fq][qr:qr + 64, q0:q0 + 1024],
                        cp[0:64, :], bc[0:64, :])

                prev = None
                for j in range(jmax + 1):
                    cur = do_scores(j)
                    if prev is not None:
                        do_ctx(prev)
                    if inject is not None and j == 8:
                        inject()
                    prev = cur
                do_ctx(prev)

            # ==== phase 1+2: load x/weights, projections + rope ====
            # (the projection tail is interleaved into attention 3a below)
            with (
                tc.tile_pool(name="proj", bufs=1) as jp,
                tc.tile_pool(name="rope", bufs=2) as rp,
            ):
                xb = jp.tile([128, KT, S], BF16, name="xb", tag="xb")
                wqb = jp.tile([128, KT, QW], BF16, name="wqb", tag="wqb")
                wkb = jp.tile([128, KT, KW], BF16, name="wkb", tag="wkb")
                wvb = jp.tile([128, KT, KW], BF16, name="wvb", tag="wvb")
                cos2t = jp.tile([128, S], BF16, name="cos2t", tag="cos2t")
                sinmt = jp.tile([128, S], BF16, name="sinmt", tag="sinmt")

                # single rearranged DMAs, ordered to match compute
                nc.sync.dma_start(
                    wkb[:], wk[:, :].rearrange("(k p) w -> p k w", p=128))
                nc.sync.dma_start(
                    xb[:, :, 0:512],
                    xT[:, 0:512].rearrange("(k p) c -> p k c", p=128))
                nc.sync.dma_start(
                    wvb[:], wv[:, :].rearrange("(k p) w -> p k w", p=128))
                nc.sync.dma_start(
                    xb[:, :, 512:1024],
                    xT[:, 512:1024].rearrange("(k p) c -> p k c", p=128))
                nc.sync.dma_start(cos2t[:], cos2[:, :])
                nc.sync.dma_start(sinmt[:], sinm[:, :])
                nc.sync.dma_start(
                    wqb[:], wq[:, :].rearrange("(k p) w -> p k w", p=128))
                nc.sync.dma_start(
                    xb[:, :, 1024:1536],
                    xT[:, 1024:1536].rearrange("(k p) c -> p k c", p=128))
                nc.sync.dma_start(
                    xb[:, :, 1536:S],
                    xT[:, 1536:S].rearrange("(k p) c -> p k c", p=128))
                nc.sync.dma_start(trit[:], msk[:, :])

                def rope_store(ps, dst, tcol):
                    # ps: psum [128, 512] f32 holding raw Q^T/K^T rows.
                    # dst[:, tcol:tcol+512] <- rope(ps) in bf16.
                    qf = rp.tile([128, 512], F32, name="ropecp", tag="ropecp")
                    nc.scalar.copy(qf[:], ps[:])
                    rot = rp.tile([128, 512], F32, name="roperot", tag="roperot")
                    for base in (0, 64):
                        nc.gpsimd.dma_start(rot[base:base + 32, :],
                                            qf[base + 32:base + 64, :])
                        nc.gpsimd.dma_start(rot[base + 32:base + 64, :],
                                            qf[base:base + 32, :])
                    a = rp.tile([128, 512], F32, name="ropea", tag="ropea")
                    b = rp.tile([128, 512], F32, name="ropeb", tag="ropeb")
                    nc.vector.tensor_mul(a[:], qf[:], cos2t[:, tcol:tcol + 512])
                    nc.vector.tensor_mul(b[:], rot[:], sinmt[:, tcol:tcol + 512])
                    nc.vector.tensor_add(dst[:, tcol:tcol + 512], a[:], b[:])

                def proj_k(t):
                    ps = st_tile()
                    for k in range(KT):
                        nc.tensor.matmul(
                            ps[:], wkb[:, k, :], xb[:, k, t * 512:(t + 1) * 512],
                            start=(k == 0), stop=(k == KT - 1))
                    rope_store(ps, Kb, t * 512)

                def proj_q(f, t):
                    ps = st_tile()
                    for k in range(KT):
                        nc.tensor.matmul(
                            ps[:], wqb[:, k, f * 128:(f + 1) * 128],
                            xb[:, k, t * 512:(t + 1) * 512],
                            start=(k == 0), stop=(k == KT - 1))
                    rope_store(ps, Qb[f], t * 512)

                def proj_v(tt):
                    # V directly in [token, feature] layout: x^T tile is the
                    # stationary operand, wv streams. out [128 tok, 128 feat].
                    ps = psp.tile([128, 128], F32, name="vp", tag="st", bufs=2,
                                  padded_shape=[128, 1024])
                    for k in range(KT):
                        nc.tensor.matmul(
                            ps[:], xb[:, k, tt * 128:(tt + 1) * 128],
                            wvb[:, k, 0:KW],
                            start=(k == 0), stop=(k == KT - 1))
                    for kv in range(KVPC):
                        nc.vector.tensor_copy(Vaug[kv][:, tt, 0:64],
                                              ps[:, kv * 64:(kv + 1) * 64])

                # everything attention 3a needs (keys/queries 0:1024):
                proj_k(0)
                for tt in range(0, 4):
                    proj_v(tt)
                proj_k(1)
                for tt in range(4, 8):
                    proj_v(tt)
                for f in range(QF):
                    proj_q(f, 0)
                for f in range(QF):
                    proj_q(f, 1)

                # remaining projection work, interleaved between 3a heads
                units = ([lambda t=t: proj_k(t) for t in (2, 3)]
                         + [lambda tt=tt: proj_v(tt) for tt in range(8, 16)]
                         + [lambda f=f, t=t: proj_q(f, t)
                            for t in (2, 3) for f in range(QF)])

                # ==== phase 3a: first query half, proj tail interleaved ====
                ui = 0
                for hl in range(HPC):
                    attn_block(hl, 0)
                    take = 2 if hl < 7 else len(units) - ui
                    for u in units[ui:ui + take]:
                        u()
                    ui += take

            # ==== phase 3b + 4: second half + output projection ====
            with tc.tile_pool(name="wout", bufs=1) as wp:
                wot = [wp.tile([128, D], BF16, name=f"wot{c}", tag=f"wot{c}")
                       for c in range(QF)]
                for c in range(QF):
                    nc.sync.dma_start(wot[c][:], wo[c * 128:(c + 1) * 128, :])

                def outproj_tile(t, tagid):
                    # one full output token-tile: 16 dense matmuls with no
                    # cross-engine deps — a long wait-free PE run that fires
                    # the HAM warm-up when injected inside attention.
                    ob = op.tile([128, D], BF16, name="ob", tag="ob")
                    ps2 = psp.tile([128, 2, 512], F32, name="ops",
                                   tag=f"cp{tagid}", bufs=1)
                    for o in range(NBL):
                        h = o % 2
                        for c in range(QF):
                            nc.tensor.matmul(
                                ps2[:, h, :],
                                ctxT[c][:, t * 128:(t + 1) * 128],
                                wot[c][:, o * 512:(o + 1) * 512],
                                start=(c == 0), stop=(c == QF - 1))
                        nc.vector.tensor_copy(ob[:, o * 512:(o + 1) * 512],
                                              ps2[:, h, :])
                    nc.gpsimd.dma_start(out[t * 128:(t + 1) * 128, :], ob[:])

                for hl in range(HPC):
                    attn_block(hl, 1,
                               inject=(lambda t=hl, g=(hl + 1) % 2:
                                       outproj_tile(t, g)))

                for t in range(8, NT):
                    outproj_tile(t, t % 2)

                if _DEBUG:
                    for f in range(QF):
                        nc.sync.dma_start(qdbg[f, :, :], Qb[f][:])
                        nc.sync.dma_start(cdbg[f, :, :], ctxT[f][:])
                    nc.sync.dma_start(kdbg[:, :], Kb[:])
                    for kv in range(KVPC):
                        nc.sync.dma_start(vdbg[kv, :, :],
                                          Vaug[kv][:, :, :])

    nc.finalize()
    return nc


def _get_nc():
    global _nc_cache
    if _nc_cache is None:
        _nc_cache = _build()
    return _nc_cache


def _prep_inputs(x, cos, sin, Wq, Wk, Wv, Wo):
    bf = ml_dtypes.bfloat16
    cosT = np.ascontiguousarray(cos.T.astype(np.float32))          # [64, S]
    sinT = sin.T.astype(np.float32)
    sinm64 = np.concatenate([-sinT[:32], sinT[32:]], axis=0)       # [64, S]
    cos2 = np.ascontiguousarray(np.concatenate([cosT, cosT], 0)).astype(bf)
    sinm = np.ascontiguousarray(np.concatenate([sinm64, sinm64], 0)).astype(bf)
    msk = (np.arange(128)[:, None] <= np.arange(128)[None, :]).astype(bf)

    # head permutation: Q^T tile f holds local heads (f, f+4) so that the
    # kv0/kv1 row base of K matches the q row base (PE base-partition rule)
    perm = [0, 4, 1, 5, 2, 6, 3, 7]
    colperm = np.concatenate(
        [np.arange(HD) + p * HD for p in perm])          # [QW]
    in_maps = []
    for c in range(NCORES):
        b, g = c // CPB, c % CPB
        xTb = np.ascontiguousarray(x[b].T.astype(bf))
        wq_g = Wq[:, g * QW:(g + 1) * QW][:, colperm]
        wo_g = Wo[g * QW:(g + 1) * QW, :][colperm, :]
        in_maps.append({
            "xT": xTb,
            "wq": np.ascontiguousarray(wq_g.astype(bf)),
            "wk": np.ascontiguousarray(Wk[:, g * KW:(g + 1) * KW].astype(bf)),
            "wv": np.ascontiguousarray(Wv[:, g * KW:(g + 1) * KW].astype(bf)),
            "wo": np.ascontiguousarray(wo_g.astype(bf)),
            "cos2": cos2,
            "sinm": sinm,
            "msk": msk,
        })
    return in_maps


def kernel(x, mask, cos, sin, Wq, Wk, Wv, Wo, _trace=False, **kw):
    x = np.asarray(x, dtype=np.float32)
    in_maps = _prep_inputs(x, np.asarray(cos), np.asarray(sin),
                           np.asarray(Wq), np.asarray(Wk),
                           np.asarray(Wv), np.asarray(Wo))
    nc = _get_nc()
    res = run_bass_kernel_spmd(nc, in_maps, core_ids=list(range(NCORES)),
                               trace=_trace, **kw)
    parts = [np.asarray(r["out"], dtype=np.float32) for r in res.results]
    full = np.stack([
        sum(parts[b * CPB + g] for g in range(CPB)) for b in range(B)
    ]).astype(np.float32)
    if _trace:
        kernel.last_result = res
    return full
